# revision 16
# baseline (speedup 1.0000x reference)
"""AdaptiveLSTMCellWithRes on 8 TRN2 NeuronCores.

Data-parallel over batch (1024 rows/core), weights replicated.
All on-chip compute happens in transposed-activation space [feat, batch]:
  - host pre-packs each weight matrix into stationary-tile slabs
    pack[j, p, k*128+m] = W[j*128+m, k*128+p]  (so W^T tiles load contiguously)
  - host pre-transposes x/h_prev/c_prev, un-transposes outputs
  - gate matmuls fuse W@x + U@h into one K=2048 accumulation over concat(x,h)
  - biases fold into the ScalarE activation that evicts PSUM
Matmuls run as float32r (full-rate fp32 on the PE array).
"""

import sys

if "/opt/trn_rl_repo" not in sys.path:
    sys.path.insert(0, "/opt/trn_rl_repo")

import numpy as np

P = 128
B = 8192          # global batch
NCORES = 8
BL = B // NCORES  # batch per core (1024)
D = 1024          # feature dim
K2 = 2048         # concat(x, h) contraction
JC = D // P       # 8 output-feature tiles
KC2 = K2 // P     # 16 k-chunks for gates/a1
KC1 = D // P      # 8 k-chunks for residual/a2
NH = BL // 2      # moving free dim per matmul (512)

_CACHE = {}


def _build():
    import concourse.bass as bass  # noqa: F401
    from concourse import bacc, mybir
    import concourse.tile as tile

    F32 = mybir.dt.float32
    MMDT = mybir.dt.float32r
    AF = mybir.ActivationFunctionType

    nc = bacc.Bacc()

    # gate weights (i, f, o, c, s, a1): packed [6, JC, P, K2]
    wg6 = nc.declare_dram_parameter("wg6", [6, JC, P, K2], MMDT, isOutput=False)
    # residual weights (r1, r2, r3): packed [3, JC, P, D]
    wr = nc.declare_dram_parameter("wr", [3, JC, P, D], MMDT, isOutput=False)
    # a2 weight: [P, KC1] with a2p[p, k] = a2_w[0, k*128+p]
    a2p = nc.declare_dram_parameter("a2p", [P, KC1], MMDT, isOutput=False)
    # biases: [P, 10*JC]; col v*JC+j holds vec_v[j*128:(j+1)*128]
    # v: 0..4 = combined gate biases (i,f,o,c,s), 5=a1_b, 6=r1_b, 7=r2_b,
    # 8=r3_b, 9=a2_b (replicated)
    biasp = nc.declare_dram_parameter("biasp", [P, 10 * JC], F32, isOutput=False)
    # transposed activations: rows 0..D-1 = x^T, D..2D-1 = h^T
    xhT = nc.declare_dram_parameter("xhT", [K2, BL], MMDT, isOutput=False)
    cT = nc.declare_dram_parameter("cT", [D, BL], F32, isOutput=False)
    # out[0] = h_t^T, out[1] = c_t^T
    out = nc.declare_dram_parameter("out", [2, D, BL], F32, isOutput=True)

    alpha_dram = nc.dram_tensor("alpha_dram", [1, BL], F32)

    GATE_FN = [AF.Sigmoid, AF.Sigmoid, AF.Sigmoid, AF.Tanh, AF.Sigmoid]

    with tile.TileContext(nc) as tc:
        with (
            tc.tile_pool(name="consts", bufs=1) as consts,
            tc.tile_pool(name="xh", bufs=1) as xh_pool,
            tc.tile_pool(name="w", bufs=4) as w_pool,
            tc.tile_pool(name="a1s", bufs=4) as a1_pool,
            tc.tile_pool(name="r1", bufs=1) as r1_pool,
            tc.tile_pool(name="r2", bufs=1) as r2_pool,
            tc.tile_pool(name="gates", bufs=1) as g_pool,
            tc.tile_pool(name="ew", bufs=2) as ew_pool,
            tc.tile_pool(name="psum", bufs=3, space="PSUM") as psum_pool,
            tc.tile_pool(name="psum_a2", bufs=1, space="PSUM") as psum_a2_pool,
        ):
            bias_sb = consts.tile([P, 10 * JC], F32, name="bias_sb")
            nc.sync.dma_start(out=bias_sb[:], in_=biasp[:, :])
            a2_sb = consts.tile([P, KC1], MMDT, name="a2_sb")
            nc.sync.dma_start(out=a2_sb[:], in_=a2p[:, :])

            def bias_ap(v, j):
                return bias_sb[:, v * JC + j: v * JC + j + 1]

            # h-half first: phase A's r1 only needs xh[KC1:]
            xh = [None] * KC2

            def load_xh(k):
                t = xh_pool.tile([P, BL], MMDT, tag=f"xh{k}", name=f"xh{k}")
                nc.sync.dma_start(out=t[:], in_=xhT[k * P:(k + 1) * P, :])
                xh[k] = t


            def mm_pair(ps2, wslabs, rhs_tiles, kc):
                # k outer / bh inner: each stationary tile feeds 2 matmuls
                for k in range(kc):
                    wt = wslabs[k // KC1]
                    kk = k % KC1
                    for bh in range(2):
                        mv = slice(bh * NH, (bh + 1) * NH)
                        nc.tensor.matmul(
                            ps2[bh][:], wt[:, kk * P:(kk + 1) * P],
                            rhs_tiles[k][:, mv],
                            start=(k == 0), stop=(k == kc - 1))

            def load_w(src_ap2, nslabs, name):
                slabs = []
                for i in range(nslabs):
                    wt = w_pool.tile([P, D], MMDT, tag="w", name=f"{name}{i}")
                    nc.sync.dma_start(out=wt[:], in_=src_ap2[:, i * D:(i + 1) * D])
                    slabs.append(wt)
                return slabs

            # first two r1 weight slabs must beat the xh stream so the PE
            # can start as soon as the first h tiles land
            r1w_pre = [load_w(wr[0, 0], 1, "wt_r1p0"),
                       load_w(wr[0, 1], 1, "wt_r1p1")]
            for k in range(KC1, KC2):
                load_xh(k)

            # dummy matmuls during the DMA head so the PE HAM clock-gate is
            # released (2.4 GHz) before real work arrives
            wslab = r1w_pre[0][0]
            wps = psum_pool.tile([P, NH], F32, tag="ps0", name="warm_ps")
            for i in range(20):
                nc.tensor.matmul(wps[:], wslab[:, :P], wslab[:, NH:2 * NH],
                                 start=(i == 0), stop=(i == 19))
            junk = a1_pool.tile([1, NH], F32, tag="a1", name="warm_junk")
            nc.scalar.activation(junk[:], wps[0:1, :], AF.Identity, bias=0.0)

            # ---- phase A: r1 (only needs h-half of xh); a1 -> a2; r2 ----
            r1 = []
            for j in range(JC):
                ws = r1w_pre[j] if j < 2 else load_w(wr[0, j], 1, "wt_r1")
                t = r1_pool.tile([P, BL], MMDT, tag=f"r1_{j}", name=f"r1_{j}")
                ps2 = [psum_pool.tile([P, NH], F32, tag="ps0", name="ps_r1_0"),
                       psum_pool.tile([P, NH], F32, tag="ps1", name="ps_r1_1")]
                mm_pair(ps2, ws, xh[KC1:], KC1)
                for bh in range(2):
                    nc.scalar.activation(t[:, bh * NH:(bh + 1) * NH], ps2[bh][:],
                                         AF.Relu, bias=bias_ap(6, j))
                r1.append(t)

            # x-half loads overlap r1 compute
            for k in range(KC1):
                load_xh(k)

            ps_a2 = [psum_a2_pool.tile([1, NH], F32, tag="a20", name="psa20"),
                     psum_a2_pool.tile([1, NH], F32, tag="a21", name="psa21")]
            pend = []

            def flush_a2():
                jq, pair = pend.pop(0)
                for bh in range(2):
                    nc.tensor.matmul(ps_a2[bh][:], a2_sb[:, jq:jq + 1],
                                     pair[bh][:], start=(jq == 0),
                                     stop=(jq == JC - 1))

            for j in range(JC):
                ws = load_w(wg6[5, j], 2, "wt_a1")
                ps2 = [psum_pool.tile([P, NH], F32, tag="ps0", name="ps_a1_0"),
                       psum_pool.tile([P, NH], F32, tag="ps1", name="ps_a1_1")]
                mm_pair(ps2, ws, xh, KC2)
                pair = []
                for bh in range(2):
                    a1b = a1_pool.tile([P, NH], MMDT, tag="a1", name="a1b")
                    nc.scalar.activation(a1b[:], ps2[bh][:], AF.Relu,
                                         bias=bias_ap(5, j))
                    pair.append(a1b)
                pend.append((j, pair))
                # defer the tiny a2 matmuls one j so PE never waits on ScalarE
                if len(pend) == 2:
                    flush_a2()
            while pend:
                flush_a2()

            r2 = []
            for j in range(JC):
                ws = load_w(wr[1, j], 1, "wt_r2")
                t = r2_pool.tile([P, BL], MMDT, tag=f"r2_{j}", name=f"r2_{j}")
                ps2 = [psum_pool.tile([P, NH], F32, tag="ps0", name="ps_r2_0"),
                       psum_pool.tile([P, NH], F32, tag="ps1", name="ps_r2_1")]
                mm_pair(ps2, ws, r1, KC1)
                for bh in range(2):
                    nc.scalar.activation(t[:, bh * NH:(bh + 1) * NH], ps2[bh][:],
                                         AF.Relu, bias=bias_ap(7, j))
                r2.append(t)

            # alpha = sigmoid(a2 @ a1relu + a2_b): [1, BL]; broadcast via DRAM
            for bh in range(2):
                asb = a1_pool.tile([1, NH], F32, tag="a1", name="alpha_sb")
                nc.scalar.activation(asb[:], ps_a2[bh][:], AF.Sigmoid,
                                     bias=bias_sb[0:1, 9 * JC: 9 * JC + 1])
                nc.sync.dma_start(out=alpha_dram[0:1, bh * NH:(bh + 1) * NH],
                                  in_=asb[:])
            alpha_rep = consts.tile([P, BL], F32, name="alpha_rep")
            nc.gpsimd.dma_start(
                out=alpha_rep[:], in_=alpha_dram[0:1, :].broadcast_to([P, BL]))

            # ---- phase B: gates + r3 + combine, per feature tile j ----
            for j in range(JC):
                g_sb = []
                for g in range(5):
                    ws = load_w(wg6[g, j], 2, f"wt_g{g}")
                    t = g_pool.tile([P, BL], F32, tag=f"g{g}", name=f"g{g}")
                    ps2 = [psum_pool.tile([P, NH], F32, tag="ps0", name="ps_g0"),
                           psum_pool.tile([P, NH], F32, tag="ps1", name="ps_g1")]
                    mm_pair(ps2, ws, xh, KC2)
                    for bh in range(2):
                        nc.scalar.activation(t[:, bh * NH:(bh + 1) * NH],
                                             ps2[bh][:], GATE_FN[g],
                                             bias=bias_ap(g, j))
                    g_sb.append(t)

                ws = load_w(wr[2, j], 1, "wt_r3")
                r3 = g_pool.tile([P, BL], F32, tag="r3", name="r3")
                ps2 = [psum_pool.tile([P, NH], F32, tag="ps0", name="ps_r3_0"),
                       psum_pool.tile([P, NH], F32, tag="ps1", name="ps_r3_1")]
                mm_pair(ps2, ws, r2, KC1)
                for bh in range(2):
                    nc.scalar.activation(r3[:, bh * NH:(bh + 1) * NH], ps2[bh][:],
                                         AF.Identity, bias=bias_ap(8, j))

                it, ft, ot, ch, st = g_sb
                NQ = NH // 2
                for q in range(4):
                    mv = slice(q * NQ, (q + 1) * NQ)
                    cp = ew_pool.tile([P, NQ], F32, tag="cp", name="cp")
                    nc.sync.dma_start(out=cp[:], in_=cT[j * P:(j + 1) * P, mv])
                    t1 = ew_pool.tile([P, NQ], F32, tag="t1", name="t1")
                    nc.vector.tensor_mul(t1[:], it[:, mv], ch[:, mv])
                    nc.vector.tensor_mul(t1[:], t1[:], st[:, mv])
                    nc.vector.tensor_mul(t1[:], t1[:], alpha_rep[:, mv])
                    t2 = ew_pool.tile([P, NQ], F32, tag="t2", name="t2")
                    nc.vector.tensor_mul(t2[:], ft[:, mv], cp[:])
                    nc.vector.tensor_add(t1[:], t1[:], t2[:])
                    nc.vector.tensor_add(t1[:], t1[:], r3[:, mv])
                    th = ew_pool.tile([P, NQ], F32, tag="th", name="th", bufs=1)
                    nc.scalar.activation(th[:], t1[:], AF.Tanh)
                    nc.vector.tensor_mul(t2[:], ot[:, mv], th[:])
                    nc.sync.dma_start(out=out[1, j * P:(j + 1) * P, mv], in_=t1[:])
                    nc.sync.dma_start(out=out[0, j * P:(j + 1) * P, mv], in_=t2[:])

    nc.finalize()
    return nc


def _pack_w(W, kdim):
    # pack[j, p, k*128+m] = W[j*128+m, k*128+p]
    kc = kdim // P
    return np.ascontiguousarray(
        W.reshape(JC, P, kc, P).transpose(0, 3, 2, 1).reshape(JC, P, kc * P))


def _prepare(inputs):
    f = lambda name: np.asarray(inputs[name], dtype=np.float32)

    gates = []
    for g in ("Wi", "Wf", "Wo", "Wc", "Ws"):
        u = "U" + g[1]
        gates.append(np.concatenate([f(g + "_w"), f(u + "_w")], axis=1))
    gates.append(f("a1_w"))
    wg6 = np.stack([_pack_w(w, K2) for w in gates])  # [6, JC, P, K2]

    wr = np.stack([_pack_w(f(n + "_w"), D) for n in ("r1", "r2", "r3")])

    a2p = np.ascontiguousarray(f("a2_w").reshape(KC1, P).T)  # [P, KC1]

    bias_vecs = []
    for g in ("Wi", "Wf", "Wo", "Wc", "Ws"):
        u = "U" + g[1]
        bias_vecs.append(f(g + "_b") + f(u + "_b"))
    bias_vecs += [f("a1_b"), f("r1_b"), f("r2_b"), f("r3_b"),
                  np.full(D, f("a2_b")[0], np.float32)]
    # biasp[p, v*JC + j] = vec_v[j*128 + p]
    biasp = np.ascontiguousarray(
        np.stack(bias_vecs).reshape(10, JC, P).transpose(2, 0, 1).reshape(P, 10 * JC))

    x, h, c = f("x"), f("h_prev"), f("c_prev")
    shared = {"wg6": wg6, "wr": wr, "a2p": a2p, "biasp": biasp}
    in_maps = []
    for core in range(NCORES):
        sl = slice(core * BL, (core + 1) * BL)
        xhT = np.ascontiguousarray(
            np.concatenate([x[sl].T, h[sl].T], axis=0))  # [K2, BL]
        cT = np.ascontiguousarray(c[sl].T)
        in_maps.append({**shared, "xhT": xhT, "cT": cT})
    return in_maps


def _run(inputs, trace=False):
    from concourse.bass_utils import run_bass_kernel_spmd

    if "nc" not in _CACHE:
        _CACHE["nc"] = _build()
    nc = _CACHE["nc"]
    in_maps = _prepare(inputs)
    res = run_bass_kernel_spmd(nc, in_maps, core_ids=list(range(NCORES)),
                               trace=trace)
    h = np.empty((B, D), np.float32)
    c = np.empty((B, D), np.float32)
    for core in range(NCORES):
        o = res.results[core]["out"]  # [2, D, BL]
        sl = slice(core * BL, (core + 1) * BL)
        h[sl] = o[0].T
        c[sl] = o[1].T
    return (h, c), res


def kernel(**inputs):
    (h, c), _ = _run(inputs, trace=False)
    return (h, c)


# revision 17
# speedup vs baseline: 1.1900x; 1.1900x over previous
"""AdaptiveLSTMCellWithRes on 8 TRN2 NeuronCores.

Data-parallel over batch (1024 rows/core), weights replicated.
All on-chip compute happens in transposed-activation space [feat, batch]:
  - host pre-packs each weight matrix into stationary-tile slabs
    pack[j, p, k*128+m] = W[j*128+m, k*128+p]  (so W^T tiles load contiguously)
  - host pre-transposes x/h_prev/c_prev, un-transposes outputs
  - gate matmuls fuse W@x + U@h into one K=2048 accumulation over concat(x,h)
  - biases fold into the ScalarE activation that evicts PSUM
Matmuls run as float32r (full-rate fp32 on the PE array).
"""

import sys

if "/opt/trn_rl_repo" not in sys.path:
    sys.path.insert(0, "/opt/trn_rl_repo")

import numpy as np

P = 128
B = 8192          # global batch
NCORES = 8
BL = B // NCORES  # batch per core (1024)
D = 1024          # feature dim
K2 = 2048         # concat(x, h) contraction
JC = D // P       # 8 output-feature tiles
KC2 = K2 // P     # 16 k-chunks for gates/a1
KC1 = D // P      # 8 k-chunks for residual/a2
NH = BL // 2      # moving free dim per matmul (512)

_CACHE = {}


def _build():
    import concourse.bass as bass  # noqa: F401
    from concourse import bacc, mybir
    import concourse.tile as tile

    F32 = mybir.dt.float32
    MMDT = mybir.dt.float32r
    AF = mybir.ActivationFunctionType

    nc = bacc.Bacc()

    # gate weights (i, f, o, c, s, a1): packed [6, JC, P, K2]
    wg6 = nc.declare_dram_parameter("wg6", [6, JC, P, K2], MMDT, isOutput=False)
    # residual weights (r1, r2, r3): packed [3, JC, P, D]
    wr = nc.declare_dram_parameter("wr", [3, JC, P, D], MMDT, isOutput=False)
    # a2 weight: [P, KC1] with a2p[p, k] = a2_w[0, k*128+p]
    a2p = nc.declare_dram_parameter("a2p", [P, KC1], MMDT, isOutput=False)
    # biases: [P, 10*JC]; col v*JC+j holds vec_v[j*128:(j+1)*128]
    # v: 0..4 = combined gate biases (i,f,o,c,s), 5=a1_b, 6=r1_b, 7=r2_b,
    # 8=r3_b, 9=a2_b (replicated)
    biasp = nc.declare_dram_parameter("biasp", [P, 10 * JC], F32, isOutput=False)
    # transposed activations: rows 0..D-1 = x^T, D..2D-1 = h^T
    xhT = nc.declare_dram_parameter("xhT", [K2, BL], MMDT, isOutput=False)
    cT = nc.declare_dram_parameter("cT", [D, BL], F32, isOutput=False)
    # out[0] = h_t^T, out[1] = c_t^T
    out = nc.declare_dram_parameter("out", [2, D, BL], F32, isOutput=True)

    alpha_dram = nc.dram_tensor("alpha_dram", [1, BL], F32)

    GATE_FN = [AF.Sigmoid, AF.Sigmoid, AF.Sigmoid, AF.Tanh, AF.Sigmoid]

    with tile.TileContext(nc) as tc:
        with (
            tc.tile_pool(name="consts", bufs=1) as consts,
            tc.tile_pool(name="xh", bufs=1) as xh_pool,
            tc.tile_pool(name="w", bufs=4) as w_pool,
            tc.tile_pool(name="a1s", bufs=4) as a1_pool,
            tc.tile_pool(name="r1", bufs=1) as r1_pool,
            tc.tile_pool(name="r2", bufs=1) as r2_pool,
            tc.tile_pool(name="gates", bufs=1) as g_pool,
            tc.tile_pool(name="ew", bufs=2) as ew_pool,
            tc.tile_pool(name="psum", bufs=3, space="PSUM") as psum_pool,
            tc.tile_pool(name="psum_a2", bufs=1, space="PSUM") as psum_a2_pool,
        ):
            bias_sb = consts.tile([P, 10 * JC], F32, name="bias_sb")
            nc.sync.dma_start(out=bias_sb[:], in_=biasp[:, :])
            a2_sb = consts.tile([P, KC1], MMDT, name="a2_sb")
            nc.sync.dma_start(out=a2_sb[:], in_=a2p[:, :])

            def bias_ap(v, j):
                return bias_sb[:, v * JC + j: v * JC + j + 1]

            # h-half first: phase A's r1 only needs xh[KC1:]
            xh = [None] * KC2

            def load_xh(k):
                t = xh_pool.tile([P, BL], MMDT, tag=f"xh{k}", name=f"xh{k}")
                nc.sync.dma_start(out=t[:], in_=xhT[k * P:(k + 1) * P, :])
                xh[k] = t


            def mm_pair(ps2, wslabs, rhs_tiles, kc):
                # k outer / bh inner: each stationary tile feeds 2 matmuls
                for k in range(kc):
                    wt = wslabs[k // KC1]
                    kk = k % KC1
                    for bh in range(2):
                        mv = slice(bh * NH, (bh + 1) * NH)
                        nc.tensor.matmul(
                            ps2[bh][:], wt[:, kk * P:(kk + 1) * P],
                            rhs_tiles[k][:, mv],
                            start=(k == 0), stop=(k == kc - 1))

            def load_w(src_ap2, nslabs, name):
                slabs = []
                for i in range(nslabs):
                    wt = w_pool.tile([P, D], MMDT, tag="w", name=f"{name}{i}")
                    nc.sync.dma_start(out=wt[:], in_=src_ap2[:, i * D:(i + 1) * D])
                    slabs.append(wt)
                return slabs

            # first two r1 weight slabs must beat the xh stream so the PE
            # can start as soon as the first h tiles land
            r1w_pre = [load_w(wr[0, 0], 1, "wt_r1p0"),
                       load_w(wr[0, 1], 1, "wt_r1p1")]
            for k in range(KC1, KC2):
                load_xh(k)

            # dummy matmuls during the DMA head so the PE HAM clock-gate is
            # released (2.4 GHz) before real work arrives
            wslab = r1w_pre[0][0]
            wps = psum_pool.tile([P, NH], F32, tag="ps0", name="warm_ps")
            for i in range(20):
                nc.tensor.matmul(wps[:], wslab[:, :P], wslab[:, NH:2 * NH],
                                 start=(i == 0), stop=(i == 19))
            junk = a1_pool.tile([1, NH], F32, tag="a1", name="warm_junk")
            nc.scalar.activation(junk[:], wps[0:1, :], AF.Identity, bias=0.0)

            # ---- phase A: r1 (only needs h-half of xh); a1 -> a2; r2 ----
            r1 = []
            for j in range(JC):
                ws = r1w_pre[j] if j < 2 else load_w(wr[0, j], 1, "wt_r1")
                t = r1_pool.tile([P, BL], MMDT, tag=f"r1_{j}", name=f"r1_{j}")
                ps2 = [psum_pool.tile([P, NH], F32, tag="ps0", name="ps_r1_0"),
                       psum_pool.tile([P, NH], F32, tag="ps1", name="ps_r1_1")]
                mm_pair(ps2, ws, xh[KC1:], KC1)
                for bh in range(2):
                    nc.scalar.activation(t[:, bh * NH:(bh + 1) * NH], ps2[bh][:],
                                         AF.Relu, bias=bias_ap(6, j))
                r1.append(t)

            # x-half loads overlap r1 compute
            for k in range(KC1):
                load_xh(k)

            ps_a2 = [psum_a2_pool.tile([1, NH], F32, tag="a20", name="psa20"),
                     psum_a2_pool.tile([1, NH], F32, tag="a21", name="psa21")]
            pend = []

            def flush_a2():
                jq, pair = pend.pop(0)
                for bh in range(2):
                    nc.tensor.matmul(ps_a2[bh][:], a2_sb[:, jq:jq + 1],
                                     pair[bh][:], start=(jq == 0),
                                     stop=(jq == JC - 1))

            for j in range(JC):
                ws = load_w(wg6[5, j], 2, "wt_a1")
                ps2 = [psum_pool.tile([P, NH], F32, tag="ps0", name="ps_a1_0"),
                       psum_pool.tile([P, NH], F32, tag="ps1", name="ps_a1_1")]
                mm_pair(ps2, ws, xh, KC2)
                pair = []
                for bh in range(2):
                    a1b = a1_pool.tile([P, NH], MMDT, tag="a1", name="a1b")
                    nc.scalar.activation(a1b[:], ps2[bh][:], AF.Relu,
                                         bias=bias_ap(5, j))
                    pair.append(a1b)
                pend.append((j, pair))
                # defer the tiny a2 matmuls one j so PE never waits on ScalarE
                if len(pend) == 2:
                    flush_a2()
            while pend:
                flush_a2()

            r2 = []
            for j in range(JC):
                ws = load_w(wr[1, j], 1, "wt_r2")
                t = r2_pool.tile([P, BL], MMDT, tag=f"r2_{j}", name=f"r2_{j}")
                ps2 = [psum_pool.tile([P, NH], F32, tag="ps0", name="ps_r2_0"),
                       psum_pool.tile([P, NH], F32, tag="ps1", name="ps_r2_1")]
                mm_pair(ps2, ws, r1, KC1)
                for bh in range(2):
                    nc.scalar.activation(t[:, bh * NH:(bh + 1) * NH], ps2[bh][:],
                                         AF.Relu, bias=bias_ap(7, j))
                r2.append(t)

            # alpha = sigmoid(a2 @ a1relu + a2_b): [1, BL]; broadcast via DRAM
            for bh in range(2):
                asb = a1_pool.tile([1, NH], F32, tag="a1", name="alpha_sb")
                nc.scalar.activation(asb[:], ps_a2[bh][:], AF.Sigmoid,
                                     bias=bias_sb[0:1, 9 * JC: 9 * JC + 1])
                nc.sync.dma_start(out=alpha_dram[0:1, bh * NH:(bh + 1) * NH],
                                  in_=asb[:])
            alpha_rep = consts.tile([P, BL], F32, name="alpha_rep")
            nc.gpsimd.dma_start(
                out=alpha_rep[:], in_=alpha_dram[0:1, :].broadcast_to([P, BL]))

            # ---- phase B: gates + r3 + combine, per feature tile j ----
            for j in range(JC):
                g_sb = []
                for g in range(5):
                    ws = load_w(wg6[g, j], 2, f"wt_g{g}")
                    t = g_pool.tile([P, BL], F32, tag=f"g{g}", name=f"g{g}")
                    ps2 = [psum_pool.tile([P, NH], F32, tag="ps0", name="ps_g0"),
                           psum_pool.tile([P, NH], F32, tag="ps1", name="ps_g1")]
                    mm_pair(ps2, ws, xh, KC2)
                    for bh in range(2):
                        nc.scalar.activation(t[:, bh * NH:(bh + 1) * NH],
                                             ps2[bh][:], GATE_FN[g],
                                             bias=bias_ap(g, j))
                    g_sb.append(t)

                ws = load_w(wr[2, j], 1, "wt_r3")
                r3 = g_pool.tile([P, BL], F32, tag="r3", name="r3")
                ps2 = [psum_pool.tile([P, NH], F32, tag="ps0", name="ps_r3_0"),
                       psum_pool.tile([P, NH], F32, tag="ps1", name="ps_r3_1")]
                mm_pair(ps2, ws, r2, KC1)
                for bh in range(2):
                    nc.scalar.activation(r3[:, bh * NH:(bh + 1) * NH], ps2[bh][:],
                                         AF.Identity, bias=bias_ap(8, j))

                it, ft, ot, ch, st = g_sb
                for bh in range(2):
                    mv = slice(bh * NH, (bh + 1) * NH)
                    cp = ew_pool.tile([P, NH], F32, tag="cp", name="cp")
                    nc.sync.dma_start(out=cp[:], in_=cT[j * P:(j + 1) * P, mv])
                    t1 = ew_pool.tile([P, NH], F32, tag="t1", name="t1")
                    nc.vector.tensor_mul(t1[:], it[:, mv], ch[:, mv])
                    nc.vector.tensor_mul(t1[:], t1[:], st[:, mv])
                    nc.vector.tensor_mul(t1[:], t1[:], alpha_rep[:, mv])
                    t2 = ew_pool.tile([P, NH], F32, tag="t2", name="t2")
                    nc.vector.tensor_mul(t2[:], ft[:, mv], cp[:])
                    nc.vector.tensor_add(t1[:], t1[:], t2[:])
                    nc.vector.tensor_add(t1[:], t1[:], r3[:, mv])
                    th = ew_pool.tile([P, NH], F32, tag="th", name="th", bufs=1)
                    nc.scalar.activation(th[:], t1[:], AF.Tanh)
                    nc.vector.tensor_mul(t2[:], ot[:, mv], th[:])
                    nc.sync.dma_start(out=out[1, j * P:(j + 1) * P, mv], in_=t1[:])
                    nc.sync.dma_start(out=out[0, j * P:(j + 1) * P, mv], in_=t2[:])

    nc.finalize()
    return nc


def _pack_w(W, kdim):
    # pack[j, p, k*128+m] = W[j*128+m, k*128+p]
    kc = kdim // P
    return np.ascontiguousarray(
        W.reshape(JC, P, kc, P).transpose(0, 3, 2, 1).reshape(JC, P, kc * P))


def _prepare(inputs):
    f = lambda name: np.asarray(inputs[name], dtype=np.float32)

    gates = []
    for g in ("Wi", "Wf", "Wo", "Wc", "Ws"):
        u = "U" + g[1]
        gates.append(np.concatenate([f(g + "_w"), f(u + "_w")], axis=1))
    gates.append(f("a1_w"))
    wg6 = np.stack([_pack_w(w, K2) for w in gates])  # [6, JC, P, K2]

    wr = np.stack([_pack_w(f(n + "_w"), D) for n in ("r1", "r2", "r3")])

    a2p = np.ascontiguousarray(f("a2_w").reshape(KC1, P).T)  # [P, KC1]

    bias_vecs = []
    for g in ("Wi", "Wf", "Wo", "Wc", "Ws"):
        u = "U" + g[1]
        bias_vecs.append(f(g + "_b") + f(u + "_b"))
    bias_vecs += [f("a1_b"), f("r1_b"), f("r2_b"), f("r3_b"),
                  np.full(D, f("a2_b")[0], np.float32)]
    # biasp[p, v*JC + j] = vec_v[j*128 + p]
    biasp = np.ascontiguousarray(
        np.stack(bias_vecs).reshape(10, JC, P).transpose(2, 0, 1).reshape(P, 10 * JC))

    x, h, c = f("x"), f("h_prev"), f("c_prev")
    shared = {"wg6": wg6, "wr": wr, "a2p": a2p, "biasp": biasp}
    in_maps = []
    for core in range(NCORES):
        sl = slice(core * BL, (core + 1) * BL)
        xhT = np.ascontiguousarray(
            np.concatenate([x[sl].T, h[sl].T], axis=0))  # [K2, BL]
        cT = np.ascontiguousarray(c[sl].T)
        in_maps.append({**shared, "xhT": xhT, "cT": cT})
    return in_maps


def _run(inputs, trace=False):
    from concourse.bass_utils import run_bass_kernel_spmd

    if "nc" not in _CACHE:
        _CACHE["nc"] = _build()
    nc = _CACHE["nc"]
    in_maps = _prepare(inputs)
    res = run_bass_kernel_spmd(nc, in_maps, core_ids=list(range(NCORES)),
                               trace=trace)
    h = np.empty((B, D), np.float32)
    c = np.empty((B, D), np.float32)
    for core in range(NCORES):
        o = res.results[core]["out"]  # [2, D, BL]
        sl = slice(core * BL, (core + 1) * BL)
        h[sl] = o[0].T
        c[sl] = o[1].T
    return (h, c), res


def kernel(**inputs):
    (h, c), _ = _run(inputs, trace=False)
    return (h, c)
